# revision 3
# baseline (speedup 1.0000x reference)
"""Paged causal GQA attention on 8 TRN2 NeuronCores.

Problem: query [8192, 32, 128] f32 (8 seqs x 1024 tokens), paged KV cache
[32 blocks, 256, 8, 128] f32, block_tables [8, 4] int32, causal attention
with GQA (32 q-heads, 8 kv-heads, n_rep=4), scale = 1/sqrt(128).

Sharding: one sequence per core. The paged-cache gather (block_tables) is
done host-side while slicing per-core inputs, so each core runs a dense
causal attention over its own 1024-token sequence. No collectives.

Per-core kernel layout (all bf16 on-chip except f32 PSUM/output):
  qT  [32, 128, 1024]  per-head Q^T (d on partitions)
  kT  [8, 128, 1024]   per-kv-head K^T (d on partitions)
  v   [8, 1024, 128]   per-kv-head V (k on partitions)
  out [1024, 32, 128]  f32

Compute per head h (kvh = h//4), with k-tiles j = 0..7 (128 keys each):
  S^T[k, q] = sum_d K^T[d,k] * Q^T[d,q]   (only q >= 128j, causal)
  causal mask inside diagonal tiles is added IN PSUM via a TensorE
  transpose of a -1e4 upper-triangular constant (start=True clears the
  bank; the score matmul then accumulates with start=False).
  P^T = exp(scale * S^T)  on ScalarE, PSUM -> SBUF bf16 (5 big ops/head)
  out[q, d] = sum_j P^T_j.T @ V'_j  accumulated in PSUM, where V' has a
  ones column appended -> column 128 of the PV output is the softmax
  denominator for free. VectorE: reciprocal + per-partition scale while
  evacuating PSUM -> SBUF f32 -> DMA out.
"""

import os
import sys

for _p in ("/opt/trn_rl_repo", "/root/.axon_site/_ro/trn_rl_repo"):
    if os.path.isdir(_p) and _p not in sys.path:
        sys.path.insert(0, _p)

import numpy as np
import ml_dtypes

BF16 = ml_dtypes.bfloat16

NUM_HEADS = 32
HEAD_DIM = 128
NUM_KV_HEADS = 8
N_REP = NUM_HEADS // NUM_KV_HEADS
SCALE = 0.08838834764831845
NUM_SEQS = 8
SEQ_LEN = 1024
NT = SEQ_LEN // 128  # 8 k/q tiles per sequence
N_CORES = 8
MASK_NEG = -1.0e4  # * SCALE = -884 -> exp underflows to exactly 0

# PSUM score-group layout. Groups of k-tiles share one 3-bank (1536 f32)
# PSUM slot so exp runs as one big ScalarE op per group.
# chunk map: j -> list of (qstart, width, group, col_in_group). The first
# chunk of every j starts at the causal diagonal (q = 128j).
CHUNKS = {
    0: [(0, 512, 0, 0), (512, 512, 0, 512)],
    1: [(128, 512, 1, 0), (640, 384, 1, 512)],
    2: [(256, 512, 2, 0), (768, 256, 2, 512)],
    3: [(384, 512, 2, 1024), (896, 128, 2, 768)],
    4: [(512, 512, 3, 0)],
    5: [(640, 384, 3, 512)],
    6: [(768, 256, 4, 0)],
    7: [(896, 128, 4, 256)],
}
GROUP_W = [1024, 896, 1536, 896, 384]  # used width per group (exp reads this)
GROUP_STRIDE = 1536  # P^T SBUF mirrors the PSUM group layout at this stride

# Per group: list of PE ops, in-order per bank so has_written semantics work:
#   ("t", j, col)                transpose of the -1e4 triangle at `col`
#                                (start=True: clears the bank's bits)
#   ("m", j, qstart, w, col, s)  score matmul chunk, start=s
# Diagonal chunks are split into a 128-wide accumulate (on top of the
# transposed mask, start=False over has_written bits) plus a remainder
# overwrite, so each matmul is uniformly accumulate-or-overwrite (the
# CoreSim PSUM model requires this; HW per-element bits agree).


def _mk_groups():
    groups = [[] for _ in GROUP_W]
    for j in sorted(CHUNKS):
        first = True
        for qs, w, g, col in CHUNKS[j]:
            ops = groups[g]
            if first:
                # diag chunk: transpose mask, accumulate 128 cols, then rest
                ops.append(("t", j, col))
                ops.append(("m", j, qs, 128, col, False))
                if w > 128:
                    ops.append(("m", j, qs + 128, w - 128, col + 128, False))
                first = False
            else:
                # standalone chunk: start=True iff first writer of its bank
                ops.append(("m", j, qs, w, col, col % 512 == 0))
    # fix cross-j bank sharing: a chunk whose bank was already written by an
    # earlier op in this group must not re-clear it (start=False), and ops
    # must be emitted in per-bank program order (they already are: j order
    # matches col order within each shared bank except G2, handled below).
    return groups


GROUPS = _mk_groups()
# G2's exp reads [0,1536) but scores only occupy [0,896)+[1024,1536): fill
# the dead 128-col hole at [896,1024) with a mask transpose so the exp input
# is always this-tile-owned data (exp of it is finite and never read).
GROUPS[2].append(("t", 3, 896))


def _pcol(j, i):
    """Column in the P^T head buffer holding q-tile i of k-tile j."""
    for qs, w, g, col in CHUNKS[j]:
        if qs <= 128 * i < qs + w:
            return g * GROUP_STRIDE + col + (128 * i - qs)
    raise AssertionError((j, i))


def _build_nc():
    import concourse.bacc as bacc
    import concourse.tile as tile
    import concourse.mybir as mybir

    f32 = mybir.dt.float32
    bf16 = mybir.dt.bfloat16
    Exp = mybir.ActivationFunctionType.Exp

    nc = bacc.Bacc("TRN2", target_bir_lowering=False, debug=False,
                   num_devices=N_CORES)

    qT = nc.dram_tensor("qT", [NUM_HEADS, HEAD_DIM, SEQ_LEN], bf16,
                        kind="ExternalInput").ap()
    kT = nc.dram_tensor("kT", [NUM_KV_HEADS, HEAD_DIM, SEQ_LEN], bf16,
                        kind="ExternalInput").ap()
    v = nc.dram_tensor("v", [NUM_KV_HEADS, SEQ_LEN, HEAD_DIM], bf16,
                       kind="ExternalInput").ap()
    cmask = nc.dram_tensor("cmask", [128, 128], f32,
                           kind="ExternalInput").ap()
    ident = nc.dram_tensor("ident", [128, 128], f32,
                           kind="ExternalInput").ap()
    out = nc.dram_tensor("out", [SEQ_LEN, NUM_HEADS, HEAD_DIM], f32,
                         kind="ExternalOutput").ap()

    with tile.TileContext(nc) as tc:
        with (
            tc.tile_pool(name="qpool", bufs=NUM_HEADS) as qpool,
            tc.tile_pool(name="kpool", bufs=NUM_KV_HEADS) as kpool,
            tc.tile_pool(name="vpool", bufs=NUM_KV_HEADS) as vpool,
            tc.tile_pool(name="cpool", bufs=1) as cpool,
            tc.tile_pool(name="ppool", bufs=2) as ppool,
            tc.tile_pool(name="opool", bufs=4) as opool,
            tc.tile_pool(name="rpool", bufs=4) as rpool,
            tc.tile_pool(name="scpool", bufs=2, space="PSUM") as scpool,
            tc.tile_pool(name="pvpool", bufs=2, space="PSUM") as pvpool,
        ):
            cm = cpool.tile([128, 128], f32, tag="cm")
            nc.sync.dma_start(out=cm[:, :], in_=cmask[:, :])
            idn = cpool.tile([128, 128], f32, tag="idn")
            nc.sync.dma_start(out=idn[:, :], in_=ident[:, :])

            kts = []
            vts = []
            for kvh in range(NUM_KV_HEADS):
                kt_t = kpool.tile([128, SEQ_LEN], bf16, tag="kt")
                nc.sync.dma_start(out=kt_t[:, :], in_=kT[kvh])
                kts.append(kt_t)
                # V' tile: 8 blocks of 129 cols (128 V cols + ones col)
                vt = vpool.tile([128, NT * 129], bf16, tag="vt")
                vt3 = vt[:, :].rearrange("p (j c) -> p j c", c=129)
                src = v[kvh].rearrange("(j p) d -> p j d", p=128)
                nc.sync.dma_start(out=vt3[:, :, 0:128], in_=src)
                nc.vector.memset(vt3[:, :, 128:129], 1.0)
                vts.append(vt)

            qts = []
            for h in range(NUM_HEADS):
                qt = qpool.tile([128, SEQ_LEN], bf16, tag="qt")
                nc.sync.dma_start(out=qt[:, :], in_=qT[h])
                qts.append(qt)

            for h in range(NUM_HEADS):
                kvh = h // N_REP
                qt, kt_t, vt = qts[h], kts[kvh], vts[kvh]

                # ---- scores S^T = K^T.T @ Q^T (+ causal -1e4 in PSUM) ----
                sc_tiles = []
                for g, ops in enumerate(GROUPS):
                    sc = scpool.tile([128, GROUP_STRIDE], f32, tag="sc")
                    for op in ops:
                        if op[0] == "t":
                            _, j, col = op
                            nc.tensor.matmul(
                                sc[:, col:col + 128], lhsT=cm[:, :],
                                rhs=idn[:, :], is_transpose=True,
                                start=True, stop=False,
                                skip_group_check=True,
                            )
                        else:
                            _, j, qs, w, col, st = op
                            nc.tensor.matmul(
                                sc[:, col:col + w],
                                lhsT=kt_t[:, 128 * j:128 * j + 128],
                                rhs=qt[:, qs:qs + w],
                                start=st, stop=True,
                                skip_group_check=True,
                            )
                    sc_tiles.append(sc)

                # ---- P^T = exp(scale * S^T), PSUM -> SBUF bf16 ----
                ph = ppool.tile([128, 5 * GROUP_STRIDE], bf16, tag="ph")
                for g, sc in enumerate(sc_tiles):
                    w = GROUP_W[g]
                    nc.scalar.activation(
                        ph[:, g * GROUP_STRIDE: g * GROUP_STRIDE + w],
                        sc[:, 0:w], Exp, scale=SCALE,
                    )

                # ---- PV: out[q, d] += P^T_j.T @ V'_j ; pairs of q-tiles
                # share one PSUM bank (even at cols 0:129, odd 129:258) ----
                for p in range(NT // 2):
                    ie, io = 2 * p, 2 * p + 1
                    pv = pvpool.tile([128, 258], f32, tag="pv")
                    for i, base, first_start in ((ie, 0, True), (io, 129, False)):
                        for j in range(i + 1):
                            c = _pcol(j, i)
                            nc.tensor.matmul(
                                pv[:, base:base + 129],
                                lhsT=ph[:, c:c + 128],
                                rhs=vt[:, 129 * j:129 * j + 129],
                                start=(j == 0 and first_start),
                                stop=(j == i),
                                skip_group_check=True,
                            )
                    # softmax denominators sit at cols 128 and 257
                    r = rpool.tile([128, 2], f32, tag="r")
                    pv3 = pv[:, :].rearrange("p (t c) -> p t c", c=129)
                    nc.vector.reciprocal(r[:, :], pv3[:, :, 128])
                    osb = opool.tile([128, 256], f32, tag="osb")
                    nc.vector.tensor_scalar_mul(
                        osb[:, 0:128], pv[:, 0:128], r[:, 0:1])
                    nc.vector.tensor_scalar_mul(
                        osb[:, 128:256], pv[:, 129:257], r[:, 1:2])
                    dst = out[256 * p:256 * p + 256, h, :].rearrange(
                        "(t q) d -> q t d", t=2)
                    src = osb[:, :].rearrange("p (t d) -> p t d", d=128)
                    nc.sync.dma_start(out=dst, in_=src)

    nc.compile()
    return nc


_NC_CACHE = {}


def _get_nc():
    if "nc" not in _NC_CACHE:
        _NC_CACHE["nc"] = _build_nc()
    return _NC_CACHE["nc"]


def make_in_maps(query, k_cache, v_cache, block_tables):
    query = np.asarray(query, dtype=np.float32)
    k_cache = np.asarray(k_cache, dtype=np.float32)
    v_cache = np.asarray(v_cache, dtype=np.float32)
    block_tables = np.asarray(block_tables)

    cmask = np.where(
        np.arange(128)[None, :] > np.arange(128)[:, None], MASK_NEG, 0.0
    ).astype(np.float32)
    ident = np.eye(128, dtype=np.float32)

    in_maps = []
    for i in range(N_CORES):
        q_i = query[SEQ_LEN * i:SEQ_LEN * (i + 1)]  # [1024, 32, 128]
        qT_i = np.ascontiguousarray(
            q_i.transpose(1, 2, 0)).astype(BF16)  # [32, 128, 1024]
        blocks = block_tables[i]
        k_i = k_cache[blocks].reshape(SEQ_LEN, NUM_KV_HEADS, HEAD_DIM)
        v_i = v_cache[blocks].reshape(SEQ_LEN, NUM_KV_HEADS, HEAD_DIM)
        kT_i = np.ascontiguousarray(k_i.transpose(1, 2, 0)).astype(BF16)
        vv_i = np.ascontiguousarray(v_i.transpose(1, 0, 2)).astype(BF16)
        in_maps.append({
            "qT": qT_i, "kT": kT_i, "v": vv_i,
            "cmask": cmask, "ident": ident,
        })
    return in_maps


def kernel(query, k_cache, v_cache, block_tables):
    from concourse.bass_utils import run_bass_kernel_spmd

    in_maps = make_in_maps(query, k_cache, v_cache, block_tables)
    nc = _get_nc()
    res = run_bass_kernel_spmd(nc, in_maps, list(range(N_CORES)))
    outs = [res.results[i]["out"] for i in range(N_CORES)]
    return np.concatenate(outs, axis=0)
